# revision 1
# baseline (speedup 1.0000x reference)
"""Trainium2 Bass kernel for nn_DiT_4758823763997 (DiT dense transformer).

B=8 batch, N=256 tokens, D=768, 12 layers, 12 heads (hd 64), MLP 3072.
Sharding: pure data-parallel — one batch element per NeuronCore (8 cores),
weights replicated; no collectives.

Device kernel (per core, one batch element):
  patch-embed GEMM -> GroupNorm(8) -> 12 x [AdaLN-mod, LN, QKV GEMM, rotary,
  attention (softmax without max-subtraction; scores are O(1)-bounded here),
  out-proj, LN, MLP with exact gelu] -> final LN -> out-proj.
Matmul operands are float32r (full-rate fp32 PE mode, ~2^-12 rounding);
accumulation fp32 in PSUM; stats/softmax math in fp32.

Host side does only tiny sidecar work: time-embedding MLP + per-layer AdaLN
shift/scale vectors (~0.25% of model FLOPs), im2col/unpatchify index
reshuffles, LN-gamma folding into adjacent GEMM weights, rotary tables.
"""

import math
import os
import sys

sys.path.insert(0, "/opt/trn_rl_repo")

import numpy as np

import concourse.bass as bass
import concourse.bacc as bacc
import concourse.mybir as mybir
import concourse.tile as tile
from concourse.bass_utils import run_bass_kernel_spmd
from concourse.masks import make_identity

B = 8
C_IN = 3
HH = 256
WW = 256
P = 16
D = 768
DEPTH = 12
NH = 12
HD = 64
MLPD = 3072
N = 256
G = 8
GS = D // G

F32 = mybir.dt.float32
# Matmul-operand dtype: "f32r" (everything float32r), "bf16" (everything
# bfloat16), or "hybrid" (MLP weights/activations bf16, rest float32r).
MM_DT_NAME = os.environ.get("KERNEL_MM_DTYPE", "f32r")
F32R = mybir.dt.bfloat16 if MM_DT_NAME == "bf16" else mybir.dt.float32r
MLPDT = mybir.dt.bfloat16 if MM_DT_NAME in ("bf16", "hybrid") else mybir.dt.float32r
AF = mybir.ActivationFunctionType
OP = mybir.AluOpType

DC = D // 128    # 6
NT = N // 128    # 2
MC = MLPD // 128  # 24

LAST_RESULT = {}
_CACHE = {}


def _ap3(ap2d, base, nblk, stride, width):
    """[128, nblk, width] free-strided view of a 2D AP at column offset base."""
    return bass.AP(tensor=ap2d.tensor, offset=ap2d.offset + base,
                   ap=[ap2d.ap[0], [stride, nblk], [1, width]])


def _row_bcast(row_ap, width, parts=128):
    """[1, W] row -> step-0 partition-broadcast AP [parts, W]."""
    return bass.AP(tensor=row_ap.tensor, offset=row_ap.offset,
                   ap=[[0, parts], [1, width]])


def _rep_free(row_ap, nrep, width):
    """[1, width] row -> [1, width*nrep] repeating each element nrep times is NOT
    this; repeats the row blockwise: index pattern [[1, width], [0, nrep]]."""
    return bass.AP(tensor=row_ap.tensor, offset=row_ap.offset,
                   ap=[row_ap.ap[0], [1, width], [0, nrep]])


def _build():
    nc = bacc.Bacc("TRN2", target_bir_lowering=False, debug=False, num_devices=8)

    def din(name, shape, dt=F32R):
        return nc.declare_dram_parameter(name, list(shape), dt, isOutput=False)

    xcolT = din("xcolT", [D, N])
    identm = din("identm", [128, 128])
    onesr = din("onesr", [1, 128])
    convw = din("convw", [D, D])
    convbr = din("convbr", [1, D])
    grow = din("grow", [1, 3 * D + 2 * G], F32)   # gn_g | gn_b | scratch
    cosn = din("cosn", [N, D], F32)
    sinsn = din("sinsn", [N, D], F32)
    Lw = []
    for i in range(DEPTH):
        Lw.append(dict(
            wqkv=din(f"wqkv{i}", [D, 3 * D]),
            wo=din(f"wo{i}", [D, D]),
            w1=din(f"w1{i}", [D, MLPD], MLPDT),
            w2=din(f"w2{i}", [MLPD, D], MLPDT),
            # shift | mod1 | bqkv(3D) | bo | b2  -> [1, 7D]
            lrow=din(f"lrow{i}", [1, 7 * D], F32),
            b1=din(f"b1{i}", [MLPD], F32),
        ))
    outw = din("outw", [D, D])
    outrow = din("outrow", [1, D], F32)
    out = nc.declare_dram_parameter("out", [N, D], F32, isOutput=True)

    with tile.TileContext(nc) as tc:
        _emit(nc, tc, xcolT, identm, onesr, convw, convbr, grow, cosn, sinsn,
              Lw, outw, outrow, out)
    nc.compile()
    return nc


def _emit(nc, tc, xcolT, identm, onesr, convw, convbr, grow, cosn, sinsn,
          Lw, outw, outrow, out):
    from contextlib import ExitStack
    with ExitStack() as ctx:
        pers = ctx.enter_context(tc.tile_pool(name="pers", bufs=1))
        wp = ctx.enter_context(tc.tile_pool(name="wp", bufs=15))
        res = ctx.enter_context(tc.tile_pool(name="res", bufs=4))
        tr = ctx.enter_context(tc.tile_pool(name="tr", bufs=5))
        wt = ctx.enter_context(tc.tile_pool(name="wt", bufs=3))
        st = ctx.enter_context(tc.tile_pool(name="st", bufs=4))
        ex = ctx.enter_context(tc.tile_pool(name="ex", bufs=3))
        ge = ctx.enter_context(tc.tile_pool(name="ge", bufs=3))
        lc = ctx.enter_context(tc.tile_pool(name="lc", bufs=2))
        ec = ctx.enter_context(tc.tile_pool(name="ec", bufs=1))
        pp = ctx.enter_context(tc.tile_pool(name="pp", bufs=8, space="PSUM"))

        ident = pers.tile([128, 128], F32R, tag="ident", name="ident")
        nc.sync.dma_start(out=ident[:], in_=identm[:, :])
        if MLPDT is F32R:
            ident_m = ident
        else:
            ident_m = pers.tile([128, 128], MLPDT, tag="identm2", name="identm2")
            nc.vector.tensor_copy(out=ident_m[:], in_=ident[:])
        ones_col = pers.tile([128, 1], F32R, tag="onesc", name="onesc")
        nc.sync.dma_start(out=ones_col[:], in_=_row_bcast(onesr[:1, :], 1))
        ones_row = pers.tile([1, 128], F32R, tag="onesr", name="onesr")
        nc.sync.dma_start(out=ones_row[:], in_=onesr[:1, :])
        eps6 = pers.tile([128, 1], F32, tag="eps6", name="eps6")
        nc.vector.memset(eps6[:], 1e-6)
        eps5 = pers.tile([128, 1], F32, tag="eps5", name="eps5")
        nc.vector.memset(eps5[:], 1e-5)

        cost = [pers.tile([128, D], F32, tag=f"cos{t}", name=f"cos{t}") for t in range(NT)]
        sint = [pers.tile([128, D], F32, tag=f"sin{t}", name=f"sin{t}") for t in range(NT)]
        for t in range(NT):
            nc.sync.dma_start(out=cost[t][:], in_=cosn[t * 128:(t + 1) * 128, :])
            nc.sync.dma_start(out=sint[t][:], in_=sinsn[t * 128:(t + 1) * 128, :])

        h = [pers.tile([128, D], F32, tag=f"h{t}", name=f"h{t}") for t in range(NT)]
        v_aug = [pers.tile([128, NH * 66], F32R, tag=f"va{t}", name=f"va{t}") for t in range(NT)]
        for t in range(NT):
            va = v_aug[t][:]
            nc.sync.dma_start(
                out=bass.AP(tensor=va.tensor, offset=va.offset + 64,
                            ap=[va.ap[0], [66, NH], [1, 2]]),
                in_=bass.AP(tensor=onesr[:1, :].tensor, offset=onesr[:1, :].offset,
                            ap=[[0, 128], [1, 2 * NH]]))

        def ln_apply(x_ap, out_ap):
            """out = (x - mean)/sqrt(var + 1e-6) along free dim 768."""
            s = st.tile([128, 16], F32, tag="lnst", name="lnst")
            nc.vector.bn_stats(out=s[:, 0:6], in_=x_ap[:, 0:384])
            nc.vector.bn_stats(out=s[:, 6:12], in_=x_ap[:, 384:768])
            sv = s[:]
            nc.vector.bn_aggr(
                out=s[:, 12:14],
                in_=bass.AP(tensor=sv.tensor, offset=sv.offset,
                            ap=[sv.ap[0], [6, 2], [1, 6]]))
            nc.scalar.activation(out=s[:, 14:15], in_=s[:, 13:14],
                                 func=AF.Sqrt, bias=eps6[:])
            nc.vector.reciprocal(out=s[:, 14:15], in_=s[:, 14:15])
            nc.vector.tensor_scalar(
                out=out_ap, in0=x_ap, scalar1=s[:, 12:13], scalar2=s[:, 14:15],
                op0=OP.subtract, op1=OP.mult)

        def transpose128(src_ap, dst_ap, dt=None):
            dt = dt or F32R
            ps = pp.tile([128, 512], dt, tag="ps", name="ps")
            nc.tensor.transpose(ps[:, 0:128], src_ap,
                                ident[:] if dt is F32R else ident_m[:])
            nc.vector.tensor_copy(out=dst_ap, in_=ps[:, 0:128])

        # ================= patch embed =================
        with nc.named_scope("embed"):
            cvb = ec.tile([1, D], F32R, tag="cvb", name="cvb")
            nc.sync.dma_start(out=cvb[:], in_=convbr[:1, :])
            ps_e = {}
            for t in range(NT):
                for js in range(2):
                    ps_e[(t, js)] = pp.tile([128, 512], F32, tag="ps", name="ps")
            for dc in range(DC):
                xt = wp.tile([128, D], F32R, tag="w", name="w")
                nc.sync.dma_start(out=xt[:, 0:N],
                                  in_=xcolT[dc * 128:(dc + 1) * 128, :])
                cwt = wp.tile([128, D], F32R, tag="w", name="w")
                nc.sync.dma_start(out=cwt[:], in_=convw[dc * 128:(dc + 1) * 128, :])
                for t in range(NT):
                    for js in range(2):
                        nc.tensor.matmul(
                            ps_e[(t, js)][:, 0:384],
                            xt[:, t * 128:(t + 1) * 128],
                            cwt[:, js * 384:(js + 1) * 384],
                            start=(dc == 0), stop=False)
            patches = [tr.tile([128, D], F32, tag="t", name="t") for _ in range(NT)]
            for t in range(NT):
                for js in range(2):
                    # + conv_b via K=1 ones-row matmul (exact)
                    nc.tensor.matmul(
                        ps_e[(t, js)][:, 0:384], ones_row[:1, :],
                        cvb[:1, js * 384:(js + 1) * 384],
                        start=False, stop=True)
                    nc.vector.tensor_copy(
                        out=patches[t][:, js * 384:(js + 1) * 384],
                        in_=ps_e[(t, js)][:, 0:384])

            # GroupNorm stats over (group channels x all tokens)
            part = [st.tile([128, 2 * G], F32R, tag="gnp", name="gnp") for _ in range(NT)]
            for t in range(NT):
                sq = tr.tile([128, D], F32, tag="t", name="t")
                nc.scalar.activation(out=sq[:], in_=patches[t][:], func=AF.Square)
                # f32r shares f32 bits; reduction still accumulates at f32.
                with nc.allow_low_precision(reason="f32r == f32 bit layout"):
                    for g in range(G):
                        nc.vector.reduce_sum(out=part[t][:, g:g + 1],
                                             in_=patches[t][:, g * GS:(g + 1) * GS],
                                             axis=mybir.AxisListType.X)
                        nc.vector.reduce_sum(out=part[t][:, G + g:G + g + 1],
                                             in_=sq[:, g * GS:(g + 1) * GS],
                                             axis=mybir.AxisListType.X)
            psg = pp.tile([128, 512], F32, tag="ps", name="ps")
            for t in range(NT):
                nc.tensor.matmul(psg[0:1, 0:2 * G], ones_col[:], part[t][:],
                                 start=(t == 0), stop=(t == NT - 1))
            gr = ec.tile([1, 3 * D + 2 * G], F32, tag="grows", name="grows")
            nc.sync.dma_start(out=gr[:], in_=grow[:1, :])
            # gr: [0:768] gn_g, [768:1536] gn_b, [1536:2304] scratch row,
            #     [2304:2320] group stats
            inv_cnt = 1.0 / (GS * N)
            nc.vector.tensor_scalar_mul(out=gr[:, 2304:2304 + 2 * G],
                                        in0=psg[0:1, 0:2 * G], scalar1=inv_cnt)
            mg = gr[:, 2304:2304 + G]
            msq = gr[:, 2304 + G:2304 + 2 * G]
            mg2 = gr[:, 1536:1536 + G]
            nc.vector.tensor_mul(out=mg2, in0=mg, in1=mg)
            nc.vector.tensor_sub(out=msq, in0=msq, in1=mg2)
            nc.scalar.activation(out=msq, in_=msq, func=AF.Sqrt, bias=eps5[0:1, :])
            nc.vector.reciprocal(out=msq, in_=msq)
            # A = rstd_g * gn_g ; B = gn_b - mean_g * A (per-group scalars)
            rsx = ec.tile([1, 2 * D], F32, tag="gscr", name="gscr")
            arow = gr[:, 1536:2304]
            for g in range(G):
                nc.vector.tensor_scalar_mul(
                    out=gr[:, 1536 + g * GS:1536 + (g + 1) * GS],
                    in0=gr[:, g * GS:(g + 1) * GS],
                    scalar1=msq[0:1, g:g + 1])
                nc.vector.tensor_scalar_mul(
                    out=rsx[:, g * GS:(g + 1) * GS],
                    in0=gr[:, 1536 + g * GS:1536 + (g + 1) * GS],
                    scalar1=mg[0:1, g:g + 1])
            nc.vector.tensor_sub(out=rsx[:, 0:D], in0=gr[:, D:2 * D],
                                 in1=rsx[:, 0:D])
            ab = ec.tile([128, 2 * D], F32, tag="gnab", name="gnab")
            nc.gpsimd.partition_broadcast(ab[:, 0:D], arow)
            nc.gpsimd.partition_broadcast(ab[:, D:2 * D], rsx[:1, 0:D])
            for t in range(NT):
                tmp = tr.tile([128, D], F32, tag="t", name="t")
                nc.vector.tensor_mul(out=tmp[:], in0=patches[t][:], in1=ab[:, 0:D])
                nc.vector.tensor_add(out=h[t][:], in0=tmp[:], in1=ab[:, D:2 * D])

        # ================= transformer layers =================
        for i in range(DEPTH):
            p = Lw[i]
            with nc.named_scope(f"layer{i}"):
                LCW = 7 * D
                lcb = lc.tile([128, LCW], F32, tag="lc", name="lc")
                nc.sync.dma_start(out=lcb[:], in_=_row_bcast(p["lrow"][:1, :], LCW))
                SHIFT = lcb[:, 0:D]
                MOD1 = lcb[:, D:2 * D]
                BQ = lcb[:, 2 * D:3 * D]
                BK = lcb[:, 3 * D:4 * D]
                BV = lcb[:, 4 * D:5 * D]
                BO = lcb[:, 5 * D:6 * D]
                B2 = lcb[:, 6 * D:7 * D]
                b1c = lc.tile([128, MC], F32, tag="b1c", name="b1c")
                b1f = p["b1"][:]
                nc.sync.dma_start(
                    out=b1c[:],
                    in_=bass.AP(tensor=b1f.tensor, offset=b1f.offset,
                                ap=[[1, 128], [128, MC]]))

                # --- AdaLN-zero modulation + LN1 ---
                hmod = [res.tile([128, D], F32, tag="res", name="res") for _ in range(NT)]
                hn = [tr.tile([128, D], F32R, tag="t", name="t") for _ in range(NT)]
                for t in range(NT):
                    tmp = tr.tile([128, D], F32, tag="t", name="t")
                    ln_apply(h[t][:], tmp[:])
                    tmp2 = tr.tile([128, D], F32, tag="t", name="t")
                    nc.vector.tensor_mul(out=tmp2[:], in0=tmp[:], in1=MOD1)
                    nc.vector.tensor_add(out=hmod[t][:], in0=tmp2[:], in1=SHIFT)
                    ln_apply(hmod[t][:], hn[t][:])

                hnT = wt.tile([128, DC * N], F32R, tag="wt", name="wt")
                for dc in range(DC):
                    for t in range(NT):
                        transpose128(hn[t][:, dc * 128:(dc + 1) * 128],
                                     hnT[:, dc * N + t * 128:dc * N + (t + 1) * 128])

                # --- Q/K: GEMM (natural) + bias + rotary + transpose ---
                rotT = {}
                for which, coff, BIAS in (("q", 0, BQ), ("k", D, BK)):
                    ps_qk = {}
                    for t in range(NT):
                        for js in range(2):
                            ps_qk[(t, js)] = pp.tile([128, 512], F32, tag="ps", name="ps")
                    for dc in range(DC):
                        w_ = wp.tile([128, D], F32R, tag="w", name="w")
                        nc.sync.dma_start(
                            out=w_[:],
                            in_=p["wqkv"][dc * 128:(dc + 1) * 128, coff:coff + D])
                        for t in range(NT):
                            for js in range(2):
                                nc.tensor.matmul(
                                    ps_qk[(t, js)][:, 0:384],
                                    hnT[:, dc * N + t * 128:dc * N + (t + 1) * 128],
                                    w_[:, js * 384:(js + 1) * 384],
                                    start=(dc == 0), stop=(dc == DC - 1))
                    rT = wt.tile([128, DC * N], F32R, tag="wt", name="wt")
                    for t in range(NT):
                        rot = tr.tile([128, D], F32R, tag="t", name="t")
                        for js in range(2):
                            psap = ps_qk[(t, js)][:, 0:384]
                            nc.vector.tensor_add(out=psap, in0=psap,
                                                 in1=BIAS[:, js * 384:(js + 1) * 384])
                            cb = js * 384
                            nc.vector.tensor_tensor(
                                out=_ap3(rot[:], cb, 6, 64, 32),
                                in0=_ap3(psap, 32, 6, 64, 32),
                                in1=_ap3(sint[t][:], cb, 6, 64, 32), op=OP.mult)
                            nc.vector.tensor_tensor(
                                out=_ap3(rot[:], cb + 32, 6, 64, 32),
                                in0=_ap3(psap, 0, 6, 64, 32),
                                in1=_ap3(sint[t][:], cb + 32, 6, 64, 32), op=OP.mult)
                            ctmp = ge.tile([128, 384], F32, tag="ct", name="ct")
                            nc.vector.tensor_mul(out=ctmp[:], in0=psap,
                                                 in1=cost[t][:, cb:cb + 384])
                            nc.vector.tensor_add(out=rot[:, cb:cb + 384],
                                                 in0=rot[:, cb:cb + 384],
                                                 in1=ctmp[:])
                        for dc in range(DC):
                            transpose128(
                                rot[:, dc * 128:(dc + 1) * 128],
                                rT[:, dc * N + t * 128:dc * N + (t + 1) * 128])
                    rotT[which] = rT

                # --- V: GEMM (natural) + bias, scattered into v_aug ---
                ps_v = {}
                for t in range(NT):
                    for js in range(2):
                        ps_v[(t, js)] = pp.tile([128, 512], F32, tag="ps", name="ps")
                for dc in range(DC):
                    w_ = wp.tile([128, D], F32R, tag="w", name="w")
                    nc.sync.dma_start(
                        out=w_[:],
                        in_=p["wqkv"][dc * 128:(dc + 1) * 128, 2 * D:3 * D])
                    for t in range(NT):
                        for js in range(2):
                            nc.tensor.matmul(
                                ps_v[(t, js)][:, 0:384],
                                hnT[:, dc * N + t * 128:dc * N + (t + 1) * 128],
                                w_[:, js * 384:(js + 1) * 384],
                                start=(dc == 0), stop=(dc == DC - 1))
                for t in range(NT):
                    for js in range(2):
                        nc.vector.tensor_tensor(
                            out=_ap3(v_aug[t][:], js * 6 * 66, 6, 66, 64),
                            in0=_ap3(ps_v[(t, js)][:, 0:384], 0, 6, 64, 64),
                            in1=_ap3(BV, js * 384, 6, 64, 64), op=OP.add)

                # --- attention per head ---
                attn = [tr.tile([128, D], F32R, tag="t", name="t") for _ in range(NT)]
                attnT = wt.tile([128, DC * N], F32R, tag="wt", name="wt")
                for hd_ in range(NH):
                    jc = hd_ // 2
                    po = (hd_ % 2) * 64
                    es = ex.tile([128, 512], F32R, tag="ex", name="ex")
                    for mc in range(NT):
                        ps = pp.tile([128, 512], F32, tag="ps", name="ps")
                        nc.tensor.matmul(
                            ps[:, 0:256],
                            rotT["k"][po:po + 64,
                                      jc * N + mc * 128:jc * N + (mc + 1) * 128],
                            rotT["q"][po:po + 64, jc * N:(jc + 1) * N],
                            start=True, stop=True)
                        nc.scalar.activation(out=es[:, mc * 256:(mc + 1) * 256],
                                             in_=ps[:, 0:256], func=AF.Exp,
                                             scale=HD ** -0.5)
                    for t in range(NT):
                        ps = pp.tile([128, 512], F32, tag="ps", name="ps")
                        for mc in range(NT):
                            nc.tensor.matmul(
                                ps[:, 0:66],
                                es[:, mc * 256 + t * 128:mc * 256 + (t + 1) * 128],
                                v_aug[mc][:, hd_ * 66:(hd_ + 1) * 66],
                                start=(mc == 0), stop=(mc == NT - 1))
                        rz = st.tile([128, 1], F32, tag="rz", name="rz")
                        nc.vector.reciprocal(out=rz[:], in_=ps[:, 64:65])
                        nc.vector.tensor_scalar_mul(
                            out=attn[t][:, hd_ * 64:(hd_ + 1) * 64],
                            in0=ps[:, 0:64], scalar1=rz[:])
                    if hd_ % 2 == 1:
                        # head pair for d-chunk jc complete -> transpose now,
                        # filling PE while ACT works on the next head's exp
                        for t in range(NT):
                            transpose128(
                                attn[t][:, jc * 128:(jc + 1) * 128],
                                attnT[:, jc * N + t * 128:jc * N + (t + 1) * 128])

                # --- out-proj + residual (res = hmod) ---
                ps_o = {}
                for t in range(NT):
                    for js in range(2):
                        ps_o[(t, js)] = pp.tile([128, 512], F32, tag="ps", name="ps")
                for dc in range(DC):
                    w_ = wp.tile([128, D], F32R, tag="w", name="w")
                    nc.sync.dma_start(out=w_[:],
                                      in_=p["wo"][dc * 128:(dc + 1) * 128, :])
                    for t in range(NT):
                        for js in range(2):
                            nc.tensor.matmul(
                                ps_o[(t, js)][:, 0:384],
                                attnT[:, dc * N + t * 128:dc * N + (t + 1) * 128],
                                w_[:, js * 384:(js + 1) * 384],
                                start=(dc == 0), stop=(dc == DC - 1))
                h1 = [res.tile([128, D], F32, tag="res", name="res") for _ in range(NT)]
                for t in range(NT):
                    for js in range(2):
                        sl = slice(js * 384, (js + 1) * 384)
                        psap = ps_o[(t, js)][:, 0:384]
                        nc.vector.tensor_add(out=psap, in0=psap, in1=hmod[t][:, sl])
                        nc.vector.tensor_add(out=h1[t][:, sl], in0=psap,
                                             in1=BO[:, sl])

                # --- MLP ---
                hn2 = [tr.tile([128, D], MLPDT, tag="t", name="t") for _ in range(NT)]
                for t in range(NT):
                    ln_apply(h1[t][:], hn2[t][:])
                hn2T = wt.tile([128, DC * N], MLPDT, tag="wt", name="wt")
                for dc in range(DC):
                    for t in range(NT):
                        transpose128(hn2[t][:, dc * 128:(dc + 1) * 128],
                                     hn2T[:, dc * N + t * 128:dc * N + (t + 1) * 128],
                                     dt=MLPDT)
                ps2 = {}
                for t in range(NT):
                    for js in range(2):
                        ps2[(t, js)] = pp.tile([128, 512], F32, tag="ps", name="ps")
                for mcq in range(4):
                    w1l = []
                    for dc in range(DC):
                        w_ = wp.tile([128, D], MLPDT, tag="w", name="w")
                        nc.sync.dma_start(
                            out=w_[:],
                            in_=p["w1"][dc * 128:(dc + 1) * 128,
                                        mcq * D:(mcq + 1) * D])
                        w1l.append(w_)
                    for ms in range(6):
                        mc = mcq * 6 + ms
                        ps1 = pp.tile([128, 512], F32, tag="ps", name="ps")
                        for dc in range(DC):
                            nc.tensor.matmul(
                                ps1[:, 0:256], w1l[dc][:, ms * 128:(ms + 1) * 128],
                                hn2T[:, dc * N:(dc + 1) * N],
                                start=(dc == 0), stop=(dc == DC - 1))
                        g_ = ge.tile([128, 256], MLPDT, tag="ge", name="ge")
                        nc.scalar.activation(out=g_[:], in_=ps1[:, 0:256],
                                             func=AF.Gelu, bias=b1c[:, mc:mc + 1])
                        w2_ = wp.tile([128, D], MLPDT, tag="w", name="w")
                        nc.sync.dma_start(out=w2_[:],
                                          in_=p["w2"][mc * 128:(mc + 1) * 128, :])
                        for t in range(NT):
                            for js in range(2):
                                nc.tensor.matmul(
                                    ps2[(t, js)][:, 0:384],
                                    g_[:, t * 128:(t + 1) * 128],
                                    w2_[:, js * 384:(js + 1) * 384],
                                    start=(mc == 0), stop=(mc == MC - 1))
                for t in range(NT):
                    for js in range(2):
                        sl = slice(js * 384, (js + 1) * 384)
                        psap = ps2[(t, js)][:, 0:384]
                        nc.vector.tensor_add(out=psap, in0=psap, in1=h1[t][:, sl])
                        nc.vector.tensor_add(out=h[t][:, sl], in0=psap,
                                             in1=B2[:, sl])

        # ================= final layer =================
        with nc.named_scope("final"):
            ob = ec.tile([128, D], F32, tag="ob", name="ob")
            nc.sync.dma_start(out=ob[:], in_=_row_bcast(outrow[:1, :], D))
            hf = [tr.tile([128, D], F32R, tag="t", name="t") for _ in range(NT)]
            for t in range(NT):
                ln_apply(h[t][:], hf[t][:])
            hfT = wt.tile([128, DC * N], F32R, tag="wt", name="wt")
            for dc in range(DC):
                for t in range(NT):
                    transpose128(hf[t][:, dc * 128:(dc + 1) * 128],
                                 hfT[:, dc * N + t * 128:dc * N + (t + 1) * 128])
            ps_f = {}
            for t in range(NT):
                for js in range(2):
                    ps_f[(t, js)] = pp.tile([128, 512], F32, tag="ps", name="ps")
            for dc in range(DC):
                w_ = wp.tile([128, D], F32R, tag="w", name="w")
                nc.sync.dma_start(out=w_[:], in_=outw[dc * 128:(dc + 1) * 128, :])
                for t in range(NT):
                    for js in range(2):
                        nc.tensor.matmul(
                            ps_f[(t, js)][:, 0:384],
                            hfT[:, dc * N + t * 128:dc * N + (t + 1) * 128],
                            w_[:, js * 384:(js + 1) * 384],
                            start=(dc == 0), stop=(dc == DC - 1))
            for t in range(NT):
                osb = tr.tile([128, D], F32, tag="t", name="t")
                for js in range(2):
                    sl = slice(js * 384, (js + 1) * 384)
                    nc.vector.tensor_add(out=osb[:, sl],
                                         in0=ps_f[(t, js)][:, 0:384], in1=ob[:, sl])
                nc.sync.dma_start(out=out[t * 128:(t + 1) * 128, :], in_=osb[:])


# ---------------------------------------------------------------- host side

def _host_prep(inputs):
    f32 = np.float32
    x = np.asarray(inputs["x"], f32)
    t = np.asarray(inputs["t"], f32)

    # time embedding + AdaLN modulation (sidecar, ~0.25% of model FLOPs)
    ts = t * 1000.0
    half = 384
    freqs = np.exp(np.arange(half, dtype=f32) * f32(-math.log(10000.0) / (half - 1)))
    e = ts[:, None] * freqs[None, :]
    temb = np.concatenate([np.sin(e), np.cos(e)], axis=-1).astype(f32)
    a = temb @ np.asarray(inputs["t_w1"], f32) + np.asarray(inputs["t_b1"], f32)
    a = (a / (1.0 + np.exp(-a))).astype(f32)  # silu
    temb = (a @ np.asarray(inputs["t_w2"], f32)
            + np.asarray(inputs["t_b2"], f32)).astype(f32)
    stemb = (temb / (1.0 + np.exp(-temb))).astype(f32)  # silu(temb)
    ada_w = np.asarray(inputs["ada_w"], f32)
    ada_b = np.asarray(inputs["ada_b"], f32)
    sc = np.einsum("bk,iko->bio", stemb, ada_w).astype(f32) + ada_b[None]
    shift = sc[:, :, :D]
    mod1 = (1.0 + sc[:, :, D:]).astype(f32)

    # im2col (transposed): xcolT[b] [(c p q), n]
    xr = x.reshape(B, C_IN, HH // P, P, WW // P, P)
    xcol = xr.transpose(0, 2, 4, 1, 3, 5).reshape(B, N, D)
    xcolT = np.ascontiguousarray(xcol.transpose(0, 2, 1))

    convw = np.ascontiguousarray(np.asarray(inputs["conv_w"], f32).reshape(D, D).T)
    convbr = np.asarray(inputs["conv_b"], f32)[None]

    grow = np.zeros((1, 3 * D + 2 * G), f32)
    grow[0, 0:D] = np.asarray(inputs["gn_g"], f32)
    grow[0, D:2 * D] = np.asarray(inputs["gn_b"], f32)

    # rotary tables (natural layout, tiled over 12 heads, sign-folded)
    inv = (10000.0 ** (-(np.arange(0, HD, 2, dtype=f32)) / HD)).astype(f32)
    f_ = np.arange(N, dtype=f32)[:, None] * inv[None, :]
    cos_t = np.cos(f_).astype(f32)
    sin_t = np.sin(f_).astype(f32)
    cosn = np.ascontiguousarray(
        np.tile(np.concatenate([cos_t, cos_t], 1), (1, NH)).astype(f32))
    sinsn = np.ascontiguousarray(
        np.tile(np.concatenate([-sin_t, sin_t], 1), (1, NH)).astype(f32))

    ln1_g = np.asarray(inputs["ln1_g"], f32)
    ln1_b = np.asarray(inputs["ln1_b"], f32)
    ln2_g = np.asarray(inputs["ln2_g"], f32)
    ln2_b = np.asarray(inputs["ln2_b"], f32)

    layers = []
    for i in range(DEPTH):
        wq = np.asarray(inputs["wq"][i], f32)
        wk = np.asarray(inputs["wk"][i], f32)
        wv = np.asarray(inputs["wv"][i], f32)
        g1 = ln1_g[i][:, None]
        wqkv = np.concatenate([g1 * wq, g1 * wk, g1 * wv], axis=1).astype(f32)
        bq = np.asarray(inputs["bq"][i], f32) + ln1_b[i] @ wq
        bk = np.asarray(inputs["bk"][i], f32) + ln1_b[i] @ wk
        bv = np.asarray(inputs["bv"][i], f32) + ln1_b[i] @ wv
        w1 = np.asarray(inputs["w1"][i], f32)
        layers.append(dict(
            wqkv=np.ascontiguousarray(wqkv),
            wo=np.ascontiguousarray(np.asarray(inputs["wo"][i], f32)),
            w1=np.ascontiguousarray((ln2_g[i][:, None] * w1).astype(f32)),
            w2=np.ascontiguousarray(np.asarray(inputs["w2"][i], f32)),
            bqkv=np.concatenate([bq, bk, bv]).astype(f32),
            bo=np.asarray(inputs["bo"][i], f32),
            b1=(np.asarray(inputs["b1"][i], f32) + ln2_b[i] @ w1).astype(f32),
            b2=np.asarray(inputs["b2"][i], f32),
        ))

    out_w = np.asarray(inputs["out_w"], f32)
    outw = np.ascontiguousarray(
        (np.asarray(inputs["fin_g"], f32)[:, None] * out_w).astype(f32))
    outrow = (np.asarray(inputs["out_b"], f32)
              + np.asarray(inputs["fin_b"], f32) @ out_w).astype(f32)[None]

    import ml_dtypes
    bfc = lambda a: np.ascontiguousarray(a.astype(ml_dtypes.bfloat16))
    if MM_DT_NAME == "bf16":
        cvt = bfc
    else:
        cvt = np.ascontiguousarray
    mlpc = bfc if MM_DT_NAME in ("bf16", "hybrid") else np.ascontiguousarray
    in_maps = []
    for b in range(B):
        m = dict(
            xcolT=cvt(xcolT[b]),
            identm=cvt(np.eye(128, dtype=f32)),
            onesr=cvt(np.ones((1, 128), f32)),
            convw=cvt(convw), convbr=cvt(convbr), grow=grow,
            cosn=cosn, sinsn=sinsn, outw=cvt(outw), outrow=outrow,
        )
        for i, L in enumerate(layers):
            m[f"wqkv{i}"] = cvt(L["wqkv"])
            m[f"wo{i}"] = cvt(L["wo"])
            m[f"w1{i}"] = mlpc(L["w1"])
            m[f"w2{i}"] = mlpc(L["w2"])
            m[f"lrow{i}"] = np.concatenate([
                shift[b, i], mod1[b, i], L["bqkv"], L["bo"], L["b2"]]).astype(
                    f32)[None]
            m[f"b1{i}"] = L["b1"]
        in_maps.append(m)
    return in_maps


def kernel(**inputs):
    if "nc" not in _CACHE:
        _CACHE["nc"] = _build()
    nc = _CACHE["nc"]
    in_maps = _host_prep(inputs)
    trace = bool(os.environ.get("KERNEL_TRACE"))
    res = run_bass_kernel_spmd(nc, in_maps, list(range(B)), trace=trace)
    LAST_RESULT["res"] = res
    out = np.empty((B, C_IN, HH, WW), np.float32)
    for b in range(B):
        o = res.results[b]["out"]  # [256, 768] = [n, (c p q)]
        out[b] = (o.reshape(16, 16, C_IN, P, P)
                  .transpose(2, 0, 3, 1, 4).reshape(C_IN, HH, WW))
    return out


if __name__ == "__main__":
    _build()
    print("build ok")

